# revision 8
# baseline (speedup 1.0000x reference)
"""CrossSpectralAttention Trainium2 kernel.

Multi-head attention over 48x48 spatial tokens: B=2, C=256, 8 heads x
head_dim 32, N=2304 tokens. Sharded over 8 NeuronCores as 2 batches x 4
head-groups (2 heads per core). Each core computes its heads' Q/K/V
projections, attention, and a partial output projection (column slice of
Wo); the host sums the 4 partials per batch.

Math notes:
- Scores s = (q.k) * d^-0.5 lie in [-7.2, 7.2] for these inputs, so the
  softmax is computed without max-subtraction: exp via ScalarE, with the
  row-sum obtained by augmenting V with a ones column in the PV matmul
  (S^T layout keeps the softmax reduction on the PE, never across
  partitions).
- All compute is fp32.
"""

import numpy as np

import concourse.bass as bass
import concourse.tile as tile
from concourse import mybir
from concourse.bass_utils import run_bass_kernel_spmd
from concourse.masks import make_identity

B = 2
C = 256
N = 2304  # 48*48
NH = 8  # total heads
HPC = 2  # heads per core
HD = 32  # head dim
GD = HPC * HD  # 64 dims per core
NC = 8  # cores
NQB = 768  # query-block size for attention
NCH = N // 128  # 18 m-chunks
SCALE = float(HD) ** -0.5

F32 = mybir.dt.float32

LAST_RESULTS = None  # BassKernelResults of the most recent run (for test.py)
_CACHED_NC = None


def _split_excess_waits(nc, max_waits=1):
    """This walrus build allows a single sync-wait per instruction; move
    excess waits onto same-engine NoOps inserted before the instruction."""
    state = {"uid": 0}

    def fix_block(b):
        i = 0
        insts = b.instructions
        while i < len(insts):
            inst = insts[i]
            for sub in getattr(inst, "blocks", None) or []:
                fix_block(sub)
            si = inst.sync_info
            if si is not None and si.on_wait and len(si.on_wait) > max_waits:
                waits = list(si.on_wait)
                keep, extra = waits[:max_waits], waits[max_waits:]
                inst.sync_info = mybir.SyncInfo(
                    on_wait=keep, on_update=list(si.on_update or [])
                )
                nops = []
                for j in range(0, len(extra), max_waits):
                    nop = mybir.InstNoOp(name=f"WSPLIT-{state['uid']}", ins=[], outs=[])
                    state["uid"] += 1
                    nop.engine = inst.engine
                    nop.sync_info = mybir.SyncInfo(
                        on_wait=extra[j : j + max_waits], on_update=[]
                    )
                    nops.append(nop)
                for k, nop in enumerate(nops):
                    insts.insert(i + k, nop)
                i += len(nops)
            i += 1

    for f in nc.m.functions:
        for b in f.blocks:
            fix_block(b)


def _pieces(total, piece):
    out = []
    o = 0
    while o < total:
        ln = min(piece, total - o)
        out.append((o, ln))
        o += ln
    return out


def build_nc(split=True):
    nc = bass.Bass()

    x_d = nc.dram_tensor("x", [C, N], F32, kind="ExternalInput")
    wq_d = nc.dram_tensor("wq_t", [C, GD], F32, kind="ExternalInput")
    wk_d = nc.dram_tensor("wk_t", [C, GD], F32, kind="ExternalInput")
    wv_d = nc.dram_tensor("wv_t", [C, GD], F32, kind="ExternalInput")
    bq_d = nc.dram_tensor("bq", [GD, 1], F32, kind="ExternalInput")
    bk_d = nc.dram_tensor("bk", [GD, 1], F32, kind="ExternalInput")
    bv_d = nc.dram_tensor("bv", [GD, 1], F32, kind="ExternalInput")
    wo_d = nc.dram_tensor("wo_t", [GD, C], F32, kind="ExternalInput")
    bo_d = nc.dram_tensor("bo", [C, 1], F32, kind="ExternalInput")
    out_d = nc.dram_tensor("out_t", [N, C], F32, kind="ExternalOutput")

    with tile.TileContext(nc) as tc:
        with (
            tc.tile_pool(name="singles", bufs=1) as singles,
            tc.tile_pool(name="expp", bufs=3) as expp,
            tc.tile_pool(name="outp", bufs=3) as outp,
        ):
            # ---- constants / inputs to SBUF ----
            ident = singles.tile([128, 128], F32)
            make_identity(nc, ident)

            x_sb = singles.tile([128, 2, N], F32)
            nc.sync.dma_start(out=x_sb, in_=x_d.rearrange("(c p) n -> p c n", p=128))

            w_sb = {}
            for name, d in (("q", wq_d), ("k", wk_d), ("v", wv_d)):
                t = singles.tile([128, 2, GD], F32, tag=f"w{name}")
                nc.sync.dma_start(out=t, in_=d.rearrange("(c p) d -> p c d", p=128))
                w_sb[name] = t
            b_sb = {}
            for name, d in (("q", bq_d), ("k", bk_d), ("v", bv_d)):
                t = singles.tile([GD, 1], F32, tag=f"b{name}")
                nc.sync.dma_start(out=t, in_=d[:, :])
                b_sb[name] = t
            # per-head Wo^T slices, both at partition base 0
            wo_h = []
            for h in range(HPC):
                t = singles.tile([HD, C], F32, name=f"wo{h}", tag=f"wo{h}")
                nc.sync.dma_start(out=t, in_=wo_d[HD * h : HD * (h + 1), :])
                wo_h.append(t)
            # bo replicated across partitions: bo_rep[p, c] = bo[c]
            bo_rep = singles.tile([128, C], F32)
            bo_bcast = bass.AP(
                tensor=bo_d, offset=0, ap=[[0, 128], [1, C]]
            )
            nc.sync.dma_start(out=bo_rep, in_=bo_bcast)

            q_sb = singles.tile([GD, N], F32, tag="q_sb")
            k_sb = singles.tile([GD, N], F32, tag="k_sb")
            v_sb = singles.tile([GD, N], F32, tag="v_sb")
            dest = {"q": q_sb, "k": k_sb, "v": v_sb}

            # ---- projections: dest = W_slice @ x + b  -> [64, N] ----
            with tc.tile_pool(name="proj_psum", bufs=4, space="PSUM") as proj_psum:
                for name in ("q", "k", "v"):
                    for off, ln in _pieces(N, 512):
                        ps = proj_psum.tile([GD, 512], F32, tag="proj")
                        nc.tensor.matmul(
                            ps[:, :ln],
                            w_sb[name][:, 0, :],
                            x_sb[:, 0, off : off + ln],
                            start=True,
                            stop=False,
                        )
                        nc.tensor.matmul(
                            ps[:, :ln],
                            w_sb[name][:, 1, :],
                            x_sb[:, 1, off : off + ln],
                            start=False,
                            stop=True,
                        )
                        nc.vector.tensor_scalar(
                            out=dest[name][:, off : off + ln],
                            in0=ps[:, :ln],
                            scalar1=b_sb[name],
                            scalar2=None,
                            op0=mybir.AluOpType.add,
                        )

            # ---- V transpose + ones augmentation ----
            # vhat[:, j, 33h:33h+33] = [V_t_h(chunk j) | 1]
            vhat = singles.tile([128, NCH, 2 * (HD + 1)], F32)
            nc.vector.memset(vhat, 1.0)
            with tc.tile_pool(name="tpsum", bufs=4, space="PSUM") as tpsum:
                for j in range(NCH):
                    tp = tpsum.tile([128, GD], F32, tag="vt")
                    nc.tensor.transpose(
                        tp, v_sb[:, 128 * j : 128 * (j + 1)], ident[:GD, :GD]
                    )
                    nc.vector.tensor_copy(
                        out=vhat[:, j, :].rearrange("p (h c) -> p h c", h=2)[
                            :, :, :HD
                        ],
                        in_=tp.rearrange("p (h c) -> p h c", h=2),
                    )

            # ---- attention (S^T layout), per head / query-block ----
            # per-head O_aug (32 dims + den row), partition base 0
            oaug_h = [
                singles.tile([HD + 1, N], F32, name=f"oaug{h}", tag=f"oaug{h}")
                for h in range(HPC)
            ]
            with (
                tc.tile_pool(name="spsum", bufs=2, space="PSUM") as spsum,
                tc.tile_pool(name="opsum", bufs=2, space="PSUM") as opsum,
            ):
                for h in range(HPC):
                    hq = slice(HD * h, HD * (h + 1))
                    vh = slice(33 * h, 33 * h + 33)
                    oh = 64 * h
                    for qoff, qln in _pieces(N, NQB):
                        o_ps = opsum.tile([HD + 1, NQB], F32, tag="o")
                        for j in range(NCH):
                            s_ps = spsum.tile([128, NQB], F32, tag="s")
                            lhsT = k_sb[hq, 128 * j : 128 * (j + 1)]
                            for poff, pln in _pieces(qln, 512):
                                nc.tensor.matmul(
                                    s_ps[:, poff : poff + pln],
                                    lhsT,
                                    q_sb[hq, qoff + poff : qoff + poff + pln],
                                    start=True,
                                    stop=True,
                                )
                            ex = expp.tile([128, NQB], F32, tag="ex")
                            nc.scalar.activation(
                                out=ex[:, :qln],
                                in_=s_ps[:, :qln],
                                func=mybir.ActivationFunctionType.Exp,
                                scale=SCALE,
                            )
                            for poff, pln in _pieces(qln, 512):
                                nc.tensor.matmul(
                                    o_ps[:, poff : poff + pln],
                                    vhat[:, j, vh],
                                    ex[:, poff : poff + pln],
                                    start=(j == 0),
                                    stop=(j == NCH - 1),
                                )
                        nc.vector.tensor_copy(
                            out=oaug_h[h][:, qoff : qoff + qln],
                            in_=o_ps[:, :qln],
                        )

            # ---- denominators -> per-chunk reciprocal -> broadcast ----
            # SBUF partition<->free transposes are done via a DRAM bounce
            # (DMA APs cannot mix partition and free dims in one descriptor).
            den_t = singles.tile([128, 2 * NCH], F32)
            inv_t = singles.tile([128, 2 * NCH], F32)
            inv_bc = [
                singles.tile([HD, N], F32, name=f"invbc{h}", tag=f"invbc{h}")
                for h in range(HPC)
            ]
            with tc.tile_pool(name="dram", bufs=1, space="DRAM") as dramp:
                den_dram = dramp.tile([2, N], F32, tag="dend")
                inv_dram = dramp.tile([2, N], F32, tag="invd")
                for h in range(HPC):
                    nc.sync.dma_start(
                        out=den_dram[h : h + 1, :],
                        in_=oaug_h[h][HD : HD + 1, :],
                    )
                    nc.sync.dma_start(
                        out=den_t[:, NCH * h : NCH * (h + 1)],
                        in_=den_dram[h : h + 1, :].rearrange(
                            "o (j p) -> (o p) j", p=128
                        ),
                    )
                nc.vector.reciprocal(out=inv_t, in_=den_t)
                for h in range(HPC):
                    nc.sync.dma_start(
                        out=inv_dram[h : h + 1, :].rearrange(
                            "o (j p) -> (o p) j", p=128
                        ),
                        in_=inv_t[:, NCH * h : NCH * (h + 1)],
                    )
                    src = inv_dram[h : h + 1, :]
                    bc = bass.AP(
                        tensor=src.tensor,
                        offset=src.offset,
                        ap=[[0, HD]] + [list(d) for d in src.ap[1:]],
                    )
                    nc.sync.dma_start(out=inv_bc[h], in_=bc)

            # ---- normalize O: on = O_unnorm * inv_den ----
            on_h = [
                singles.tile([HD, N], F32, name=f"on{h}", tag=f"on{h}")
                for h in range(HPC)
            ]
            for h in range(HPC):
                nc.vector.tensor_mul(
                    out=on_h[h],
                    in0=oaug_h[h][:HD, :],
                    in1=inv_bc[h],
                )

            # ---- output projection: out_t[n, co] = on^T @ wo_t + bo ----
            with tc.tile_pool(name="wopsum", bufs=4, space="PSUM") as wopsum:
                for j in range(NCH):
                    wp = wopsum.tile([128, C], F32, tag="wo")
                    for h in range(HPC):
                        nc.tensor.matmul(
                            wp,
                            on_h[h][:, 128 * j : 128 * (j + 1)],
                            wo_h[h],
                            start=(h == 0),
                            stop=(h == HPC - 1),
                        )
                    ot = outp.tile([128, C], F32, tag="ot")
                    nc.vector.tensor_add(out=ot, in0=wp, in1=bo_rep)
                    nc.sync.dma_start(
                        out=out_d[128 * j : 128 * (j + 1), :], in_=ot
                    )

    if split:
        _split_excess_waits(nc)
    return nc


def kernel(x, Wq, bq, Wk, bk, Wv, bv, Wo, bo):
    global LAST_RESULTS, _CACHED_NC
    x = np.ascontiguousarray(np.asarray(x, dtype=np.float32))
    Wq = np.asarray(Wq, dtype=np.float32)
    Wk = np.asarray(Wk, dtype=np.float32)
    Wv = np.asarray(Wv, dtype=np.float32)
    Wo = np.asarray(Wo, dtype=np.float32)
    bq = np.asarray(bq, dtype=np.float32)
    bk = np.asarray(bk, dtype=np.float32)
    bv = np.asarray(bv, dtype=np.float32)
    bo = np.asarray(bo, dtype=np.float32)

    xf = x.reshape(B, C, N)
    in_maps = []
    for core in range(NC):
        b = core // 4
        g = core % 4
        sl = slice(GD * g, GD * (g + 1))
        in_maps.append(
            {
                "x": np.ascontiguousarray(xf[b]),
                "wq_t": np.ascontiguousarray(Wq[sl, :].T),
                "wk_t": np.ascontiguousarray(Wk[sl, :].T),
                "wv_t": np.ascontiguousarray(Wv[sl, :].T),
                "bq": np.ascontiguousarray(bq[sl].reshape(GD, 1)),
                "bk": np.ascontiguousarray(bk[sl].reshape(GD, 1)),
                "bv": np.ascontiguousarray(bv[sl].reshape(GD, 1)),
                "wo_t": np.ascontiguousarray(Wo[:, sl].T),
                "bo": np.ascontiguousarray(
                    bo.reshape(C, 1) if g == 0 else np.zeros((C, 1), np.float32)
                ),
            }
        )

    if _CACHED_NC is None:
        _CACHED_NC = build_nc()
    res = run_bass_kernel_spmd(_CACHED_NC, in_maps, core_ids=list(range(NC)))
    LAST_RESULTS = res

    out = np.zeros((B, C, N), dtype=np.float32)
    for core in range(NC):
        out[core // 4] += res.results[core]["out_t"].T
    return out.reshape(B, C, 48, 48)


# revision 13
# speedup vs baseline: 1.6275x; 1.6275x over previous
"""CrossSpectralAttention Trainium2 kernel.

Multi-head attention over 48x48 spatial tokens: B=2, C=256, 8 heads x
head_dim 32, N=2304 tokens. Sharded over 8 NeuronCores as 2 batches x 4
head-groups (2 heads per core). Each core computes its heads' Q/K/V
projections, attention, and a partial output projection (column slice of
Wo); the host sums the 4 partials per batch.

Math notes:
- Scores s = (q.k) * d^-0.5 lie in [-7.2, 7.2] for these inputs, so the
  softmax is computed without max-subtraction: exp via ScalarE, with the
  row-sum obtained by augmenting V with a ones column in the PV matmul
  (S^T layout keeps the softmax reduction on the PE, never across
  partitions).
- All compute is fp32.
"""

import numpy as np

import concourse.bass as bass
import concourse.tile as tile
from concourse import mybir
from concourse.bass_utils import run_bass_kernel_spmd

B = 2
C = 256
N = 2304  # 48*48
NH = 8  # total heads
HPC = 2  # heads per core
HD = 32  # head dim
GD = HPC * HD  # 64 dims per core
NC = 8  # cores
NQB = 768  # query-block size for attention
NCH = N // 128  # 18 m-chunks
SCALE = float(HD) ** -0.5

F32 = mybir.dt.float32
# float32r: fp32 storage, single-pass PE matmul (4x fp32 throughput); any
# instruction producing a matmul operand must itself write float32r.
F32R = mybir.dt.float32r

LAST_RESULTS = None  # BassKernelResults of the most recent run (for test.py)
_CACHED_NC = None


def _split_excess_waits(nc, max_waits=1):
    """This walrus build allows a single sync-wait per instruction; move
    excess waits onto same-engine NoOps inserted before the instruction."""
    state = {"uid": 0}

    def fix_block(b):
        i = 0
        insts = b.instructions
        while i < len(insts):
            inst = insts[i]
            for sub in getattr(inst, "blocks", None) or []:
                fix_block(sub)
            si = inst.sync_info
            if si is not None and si.on_wait and len(si.on_wait) > max_waits:
                waits = list(si.on_wait)
                keep, extra = waits[:max_waits], waits[max_waits:]
                inst.sync_info = mybir.SyncInfo(
                    on_wait=keep, on_update=list(si.on_update or [])
                )
                nops = []
                for j in range(0, len(extra), max_waits):
                    nop = mybir.InstNoOp(name=f"WSPLIT-{state['uid']}", ins=[], outs=[])
                    state["uid"] += 1
                    nop.engine = inst.engine
                    nop.sync_info = mybir.SyncInfo(
                        on_wait=extra[j : j + max_waits], on_update=[]
                    )
                    nops.append(nop)
                for k, nop in enumerate(nops):
                    insts.insert(i + k, nop)
                i += len(nops)
            i += 1

    for f in nc.m.functions:
        for b in f.blocks:
            fix_block(b)


def _pieces(total, piece):
    out = []
    o = 0
    while o < total:
        ln = min(piece, total - o)
        out.append((o, ln))
        o += ln
    return out


def build_nc(split=True):
    nc = bass.Bass()

    x_d = nc.dram_tensor("x", [C, N], F32R, kind="ExternalInput")
    wq_d = nc.dram_tensor("wq_t", [C, GD], F32R, kind="ExternalInput")
    wk_d = nc.dram_tensor("wk_t", [C, GD], F32R, kind="ExternalInput")
    wv_d = nc.dram_tensor("wv_t", [C, GD], F32R, kind="ExternalInput")
    bq_d = nc.dram_tensor("bq", [GD, 1], F32, kind="ExternalInput")
    bk_d = nc.dram_tensor("bk", [GD, 1], F32, kind="ExternalInput")
    bv_d = nc.dram_tensor("bv", [GD, 1], F32, kind="ExternalInput")
    wo_d = nc.dram_tensor("wo_t", [GD, C], F32R, kind="ExternalInput")
    bo_d = nc.dram_tensor("bo", [C, 1], F32, kind="ExternalInput")
    out_d = nc.dram_tensor("out_t", [N, C], F32, kind="ExternalOutput")

    with tile.TileContext(nc) as tc:
        with (
            tc.tile_pool(name="singles", bufs=1) as singles,
            tc.tile_pool(name="expp", bufs=3) as expp,
            tc.tile_pool(name="outp", bufs=3) as outp,
        ):
            # ---- constants / inputs to SBUF ----
            x_sb = singles.tile([128, 2, N], F32R)
            nc.sync.dma_start(out=x_sb, in_=x_d.rearrange("(c p) n -> p c n", p=128))

            w_sb = {}
            for name, d in (("q", wq_d), ("k", wk_d), ("v", wv_d)):
                t = singles.tile([128, 2, GD], F32R, tag=f"w{name}")
                nc.sync.dma_start(out=t, in_=d.rearrange("(c p) d -> p c d", p=128))
                w_sb[name] = t
            b_sb = {}
            for name, d in (("q", bq_d), ("k", bk_d)):
                t = singles.tile([GD, 1], F32, tag=f"b{name}")
                nc.sync.dma_start(out=t, in_=d[:, :])
                b_sb[name] = t
            # bv replicated across partitions for the V^T layout bias add
            bv_rep = singles.tile([128, GD], F32)
            nc.sync.dma_start(
                out=bv_rep,
                in_=bass.AP(tensor=bv_d, offset=0, ap=[[0, 128], [1, GD]]),
            )
            # per-head Wo^T slices, both at partition base 0
            wo_h = []
            for h in range(HPC):
                t = singles.tile([HD, C], F32R, name=f"wo{h}", tag=f"wo{h}")
                nc.sync.dma_start(out=t, in_=wo_d[HD * h : HD * (h + 1), :])
                wo_h.append(t)
            # bo replicated across partitions: bo_rep[p, c] = bo[c]
            bo_rep = singles.tile([128, C], F32)
            bo_bcast = bass.AP(
                tensor=bo_d, offset=0, ap=[[0, 128], [1, C]]
            )
            nc.sync.dma_start(out=bo_rep, in_=bo_bcast)

            q_sb = singles.tile([GD, N], F32R, tag="q_sb")
            k_sb = singles.tile([GD, N], F32R, tag="k_sb")
            dest = {"q": q_sb, "k": k_sb}

            # ---- projections: dest = W_slice @ x + b  -> [64, N] ----
            with tc.tile_pool(name="proj_psum", bufs=4, space="PSUM") as proj_psum:
                for name in ("q", "k"):
                    for off, ln in _pieces(N, 512):
                        ps = proj_psum.tile([GD, 512], F32, tag="proj")
                        nc.tensor.matmul(
                            ps[:, :ln],
                            w_sb[name][:, 0, :],
                            x_sb[:, 0, off : off + ln],
                            start=True,
                            stop=False,
                        )
                        nc.tensor.matmul(
                            ps[:, :ln],
                            w_sb[name][:, 1, :],
                            x_sb[:, 1, off : off + ln],
                            start=False,
                            stop=True,
                        )
                        nc.vector.tensor_scalar(
                            out=dest[name][:, off : off + ln],
                            in0=ps[:, :ln],
                            scalar1=b_sb[name],
                            scalar2=None,
                            op0=mybir.AluOpType.add,
                        )

            # ---- V^T directly: vhat[:, j, 33h:33h+33] = [V_t_h(chunk j) | 1]
            # V_t chunk [n=128, dv] = x_chunk^T @ Wv^T (+ bv broadcast)
            vhat = singles.tile([128, NCH, 2 * (HD + 1)], F32R)
            ones2 = singles.tile([128, 2], F32)
            nc.vector.memset(ones2, 1.0)
            with tc.tile_pool(name="tpsum", bufs=4, space="PSUM") as tpsum:
                for j in range(NCH):
                    tp = tpsum.tile([128, GD], F32, tag="vt")
                    for c in range(2):
                        nc.tensor.matmul(
                            tp,
                            x_sb[:, c, 128 * j : 128 * (j + 1)],
                            w_sb["v"][:, c, :],
                            start=(c == 0),
                            stop=(c == 1),
                        )
                    nc.vector.tensor_tensor(
                        out=vhat[:, j, :].rearrange("p (h c) -> p h c", h=2)[
                            :, :, :HD
                        ],
                        in0=tp.rearrange("p (h c) -> p h c", h=2),
                        in1=bv_rep.rearrange("p (h c) -> p h c", h=2),
                        op=mybir.AluOpType.add,
                    )
                    nc.vector.tensor_copy(
                        out=vhat[:, j, :].rearrange("p (h c) -> p h c", h=2)[
                            :, :, HD : HD + 1
                        ],
                        in_=ones2.rearrange("p (h c) -> p h c", h=2),
                    )

            # ---- attention (S^T layout), per head / query-block ----
            # per-head O_aug (32 dims + den row), partition base 0
            oaug_h = [
                singles.tile([HD + 1, N], F32, name=f"oaug{h}", tag=f"oaug{h}")
                for h in range(HPC)
            ]
            with (
                tc.tile_pool(name="spsum", bufs=2, space="PSUM") as spsum,
                tc.tile_pool(name="opsum", bufs=2, space="PSUM") as opsum,
            ):
                for h in range(HPC):
                    hq = slice(HD * h, HD * (h + 1))
                    vh = slice(33 * h, 33 * h + 33)
                    oh = 64 * h
                    for qoff, qln in _pieces(N, NQB):
                        o_ps = opsum.tile([HD + 1, NQB], F32, tag="o")
                        for j in range(NCH):
                            s_ps = spsum.tile([128, NQB], F32, tag="s")
                            lhsT = k_sb[hq, 128 * j : 128 * (j + 1)]
                            for poff, pln in _pieces(qln, 512):
                                nc.tensor.matmul(
                                    s_ps[:, poff : poff + pln],
                                    lhsT,
                                    q_sb[hq, qoff + poff : qoff + poff + pln],
                                    start=True,
                                    stop=True,
                                )
                            ex = expp.tile([128, NQB], F32R, tag="ex")
                            nc.scalar.activation(
                                out=ex[:, :qln],
                                in_=s_ps[:, :qln],
                                func=mybir.ActivationFunctionType.Exp,
                                scale=SCALE,
                            )
                            for poff, pln in _pieces(qln, 512):
                                nc.tensor.matmul(
                                    o_ps[:, poff : poff + pln],
                                    vhat[:, j, vh],
                                    ex[:, poff : poff + pln],
                                    start=(j == 0),
                                    stop=(j == NCH - 1),
                                )
                        nc.vector.tensor_copy(
                            out=oaug_h[h][:, qoff : qoff + qln],
                            in_=o_ps[:, :qln],
                        )

            # ---- denominators -> per-chunk reciprocal -> broadcast ----
            # SBUF partition<->free transposes are done via a DRAM bounce
            # (DMA APs cannot mix partition and free dims in one descriptor).
            den_t = singles.tile([128, 2 * NCH], F32)
            inv_t = singles.tile([128, 2 * NCH], F32)
            inv_bc = [
                singles.tile([HD, N], F32, name=f"invbc{h}", tag=f"invbc{h}")
                for h in range(HPC)
            ]
            with tc.tile_pool(name="dram", bufs=1, space="DRAM") as dramp:
                den_dram = dramp.tile([2, N], F32, tag="dend")
                inv_dram = dramp.tile([2, N], F32, tag="invd")
                for h in range(HPC):
                    nc.sync.dma_start(
                        out=den_dram[h : h + 1, :],
                        in_=oaug_h[h][HD : HD + 1, :],
                    )
                    nc.sync.dma_start(
                        out=den_t[:, NCH * h : NCH * (h + 1)],
                        in_=den_dram[h : h + 1, :].rearrange(
                            "o (j p) -> (o p) j", p=128
                        ),
                    )
                nc.vector.reciprocal(out=inv_t, in_=den_t)
                for h in range(HPC):
                    nc.sync.dma_start(
                        out=inv_dram[h : h + 1, :].rearrange(
                            "o (j p) -> (o p) j", p=128
                        ),
                        in_=inv_t[:, NCH * h : NCH * (h + 1)],
                    )
                    src = inv_dram[h : h + 1, :]
                    bc = bass.AP(
                        tensor=src.tensor,
                        offset=src.offset,
                        ap=[[0, HD]] + [list(d) for d in src.ap[1:]],
                    )
                    nc.sync.dma_start(out=inv_bc[h], in_=bc)

            # ---- normalize O: on = O_unnorm * inv_den ----
            on_h = [
                singles.tile([HD, N], F32R, name=f"on{h}", tag=f"on{h}")
                for h in range(HPC)
            ]
            for h in range(HPC):
                nc.vector.tensor_mul(
                    out=on_h[h],
                    in0=oaug_h[h][:HD, :],
                    in1=inv_bc[h],
                )

            # ---- output projection: out_t[n, co] = on^T @ wo_t + bo ----
            with tc.tile_pool(name="wopsum", bufs=4, space="PSUM") as wopsum:
                for j in range(NCH):
                    wp = wopsum.tile([128, C], F32, tag="wo")
                    for h in range(HPC):
                        nc.tensor.matmul(
                            wp,
                            on_h[h][:, 128 * j : 128 * (j + 1)],
                            wo_h[h],
                            start=(h == 0),
                            stop=(h == HPC - 1),
                        )
                    ot = outp.tile([128, C], F32, tag="ot")
                    nc.vector.tensor_add(out=ot, in0=wp, in1=bo_rep)
                    nc.sync.dma_start(
                        out=out_d[128 * j : 128 * (j + 1), :], in_=ot
                    )

    if split:
        _split_excess_waits(nc)
    return nc


def kernel(x, Wq, bq, Wk, bk, Wv, bv, Wo, bo):
    global LAST_RESULTS, _CACHED_NC
    x = np.ascontiguousarray(np.asarray(x, dtype=np.float32))
    Wq = np.asarray(Wq, dtype=np.float32)
    Wk = np.asarray(Wk, dtype=np.float32)
    Wv = np.asarray(Wv, dtype=np.float32)
    Wo = np.asarray(Wo, dtype=np.float32)
    bq = np.asarray(bq, dtype=np.float32)
    bk = np.asarray(bk, dtype=np.float32)
    bv = np.asarray(bv, dtype=np.float32)
    bo = np.asarray(bo, dtype=np.float32)

    xf = x.reshape(B, C, N)
    in_maps = []
    for core in range(NC):
        b = core // 4
        g = core % 4
        sl = slice(GD * g, GD * (g + 1))
        in_maps.append(
            {
                "x": np.ascontiguousarray(xf[b]),
                "wq_t": np.ascontiguousarray(Wq[sl, :].T),
                "wk_t": np.ascontiguousarray(Wk[sl, :].T),
                "wv_t": np.ascontiguousarray(Wv[sl, :].T),
                "bq": np.ascontiguousarray(bq[sl].reshape(GD, 1)),
                "bk": np.ascontiguousarray(bk[sl].reshape(GD, 1)),
                "bv": np.ascontiguousarray(bv[sl].reshape(GD, 1)),
                "wo_t": np.ascontiguousarray(Wo[:, sl].T),
                "bo": np.ascontiguousarray(
                    bo.reshape(C, 1) if g == 0 else np.zeros((C, 1), np.float32)
                ),
            }
        )

    if _CACHED_NC is None:
        _CACHED_NC = build_nc()
    res = run_bass_kernel_spmd(_CACHED_NC, in_maps, core_ids=list(range(NC)))
    LAST_RESULTS = res

    out = np.zeros((B, C, N), dtype=np.float32)
    for core in range(NC):
        out[core // 4] += res.results[core]["out_t"].T
    return out.reshape(B, C, 48, 48)


# revision 14
# speedup vs baseline: 2.3849x; 1.4654x over previous
"""CrossSpectralAttention Trainium2 kernel.

Multi-head attention over 48x48 spatial tokens: B=2, C=256, 8 heads x
head_dim 32, N=2304 tokens. Sharded over 8 NeuronCores as 2 batches x 4
head-groups (2 heads per core). Each core computes its heads' Q/K/V
projections, attention, and a partial output projection (column slice of
Wo); the host sums the 4 partials per batch.

Math notes:
- Scores s = (q.k) * d^-0.5 lie in [-7.2, 7.2] for these inputs, so the
  softmax is computed without max-subtraction: exp via ScalarE, with the
  row-sum obtained by augmenting V with a ones column in the PV matmul
  (S^T layout keeps the softmax reduction on the PE, never across
  partitions).
- All compute is fp32.
"""

import numpy as np

import concourse.bass as bass
import concourse.tile as tile
from concourse import mybir
from concourse.bass_utils import run_bass_kernel_spmd

B = 2
C = 256
N = 2304  # 48*48
NH = 8  # total heads
HPC = 2  # heads per core
HD = 32  # head dim
GD = HPC * HD  # 64 dims per core
NC = 8  # cores
NQB = 512  # query-block size for attention
NCH = N // 128  # 18 m-chunks
SCALE = float(HD) ** -0.5

F32 = mybir.dt.float32
# float32r: fp32 storage, single-pass PE matmul (4x fp32 throughput); any
# instruction producing a matmul operand must itself write float32r.
F32R = mybir.dt.float32r

LAST_RESULTS = None  # BassKernelResults of the most recent run (for test.py)
_CACHED_NC = None


def _split_excess_waits(nc, max_waits=1):
    """This walrus build allows a single sync-wait per instruction; move
    excess waits onto same-engine NoOps inserted before the instruction."""
    state = {"uid": 0}

    def fix_block(b):
        i = 0
        insts = b.instructions
        while i < len(insts):
            inst = insts[i]
            for sub in getattr(inst, "blocks", None) or []:
                fix_block(sub)
            si = inst.sync_info
            if si is not None and si.on_wait and len(si.on_wait) > max_waits:
                waits = list(si.on_wait)
                keep, extra = waits[:max_waits], waits[max_waits:]
                inst.sync_info = mybir.SyncInfo(
                    on_wait=keep, on_update=list(si.on_update or [])
                )
                nops = []
                for j in range(0, len(extra), max_waits):
                    nop = mybir.InstNoOp(name=f"WSPLIT-{state['uid']}", ins=[], outs=[])
                    state["uid"] += 1
                    nop.engine = inst.engine
                    nop.sync_info = mybir.SyncInfo(
                        on_wait=extra[j : j + max_waits], on_update=[]
                    )
                    nops.append(nop)
                for k, nop in enumerate(nops):
                    insts.insert(i + k, nop)
                i += len(nops)
            i += 1

    for f in nc.m.functions:
        for b in f.blocks:
            fix_block(b)


def _pieces(total, piece):
    out = []
    o = 0
    while o < total:
        ln = min(piece, total - o)
        out.append((o, ln))
        o += ln
    return out


def build_nc(split=True):
    nc = bass.Bass()

    x_d = nc.dram_tensor("x", [C, N], F32R, kind="ExternalInput")
    wq_d = nc.dram_tensor("wq_t", [C, GD], F32R, kind="ExternalInput")
    wk_d = nc.dram_tensor("wk_t", [C, GD], F32R, kind="ExternalInput")
    wv_d = nc.dram_tensor("wv_t", [C, GD], F32R, kind="ExternalInput")
    bq_d = nc.dram_tensor("bq", [GD, 1], F32, kind="ExternalInput")
    bk_d = nc.dram_tensor("bk", [GD, 1], F32, kind="ExternalInput")
    bv_d = nc.dram_tensor("bv", [GD, 1], F32, kind="ExternalInput")
    wo_d = nc.dram_tensor("wo_t", [GD, C], F32R, kind="ExternalInput")
    bo_d = nc.dram_tensor("bo", [C, 1], F32, kind="ExternalInput")
    out_d = nc.dram_tensor("out_t", [N, C], F32, kind="ExternalOutput")

    with tile.TileContext(nc) as tc:
        with (
            tc.tile_pool(name="singles", bufs=1) as singles,
            tc.tile_pool(name="expp", bufs=3) as expp,
            tc.tile_pool(name="outp", bufs=3) as outp,
        ):
            # ---- constants / inputs to SBUF ----
            x_sb = singles.tile([128, 2, N], F32R)
            nc.sync.dma_start(out=x_sb, in_=x_d.rearrange("(c p) n -> p c n", p=128))

            w_sb = {}
            for name, d in (("q", wq_d), ("k", wk_d), ("v", wv_d)):
                t = singles.tile([128, 2, GD], F32R, tag=f"w{name}")
                nc.sync.dma_start(out=t, in_=d.rearrange("(c p) d -> p c d", p=128))
                w_sb[name] = t
            b_sb = {}
            for name, d in (("q", bq_d), ("k", bk_d)):
                t = singles.tile([GD, 1], F32, tag=f"b{name}")
                nc.sync.dma_start(out=t, in_=d[:, :])
                b_sb[name] = t
            # bv replicated across partitions for the V^T layout bias add
            bv_rep = singles.tile([128, GD], F32)
            nc.sync.dma_start(
                out=bv_rep,
                in_=bass.AP(tensor=bv_d, offset=0, ap=[[0, 128], [1, GD]]),
            )
            # per-head Wo^T slices, both at partition base 0
            wo_h = []
            for h in range(HPC):
                t = singles.tile([HD, C], F32R, name=f"wo{h}", tag=f"wo{h}")
                nc.sync.dma_start(out=t, in_=wo_d[HD * h : HD * (h + 1), :])
                wo_h.append(t)
            # bo replicated across partitions: bo_rep[p, c] = bo[c]
            bo_rep = singles.tile([128, C], F32)
            bo_bcast = bass.AP(
                tensor=bo_d, offset=0, ap=[[0, 128], [1, C]]
            )
            nc.sync.dma_start(out=bo_rep, in_=bo_bcast)

            q_sb = singles.tile([GD, N], F32R, tag="q_sb")
            k_sb = singles.tile([GD, N], F32R, tag="k_sb")
            dest = {"q": q_sb, "k": k_sb}

            # ---- projections: dest = W_slice @ x + b  -> [64, N] ----
            with tc.tile_pool(name="proj_psum", bufs=4, space="PSUM") as proj_psum:
                for name in ("q", "k"):
                    for off, ln in _pieces(N, 512):
                        ps = proj_psum.tile([GD, 512], F32, tag="proj")
                        nc.tensor.matmul(
                            ps[:, :ln],
                            w_sb[name][:, 0, :],
                            x_sb[:, 0, off : off + ln],
                            start=True,
                            stop=False,
                        )
                        nc.tensor.matmul(
                            ps[:, :ln],
                            w_sb[name][:, 1, :],
                            x_sb[:, 1, off : off + ln],
                            start=False,
                            stop=True,
                        )
                        nc.vector.tensor_scalar(
                            out=dest[name][:, off : off + ln],
                            in0=ps[:, :ln],
                            scalar1=b_sb[name],
                            scalar2=None,
                            op0=mybir.AluOpType.add,
                        )

            # ---- V^T directly: vhat[:, j, 33h:33h+33] = [V_t_h(chunk j) | 1]
            # V_t chunk [n=128, dv] = x_chunk^T @ Wv^T (+ bv broadcast)
            vhat = singles.tile([128, NCH, 2 * (HD + 1)], F32R)
            ones2 = singles.tile([128, 2], F32)
            nc.vector.memset(ones2, 1.0)
            with tc.tile_pool(name="tpsum", bufs=4, space="PSUM") as tpsum:
                for j in range(NCH):
                    tp = tpsum.tile([128, GD], F32, tag="vt")
                    for c in range(2):
                        nc.tensor.matmul(
                            tp,
                            x_sb[:, c, 128 * j : 128 * (j + 1)],
                            w_sb["v"][:, c, :],
                            start=(c == 0),
                            stop=(c == 1),
                        )
                    nc.vector.tensor_tensor(
                        out=vhat[:, j, :].rearrange("p (h c) -> p h c", h=2)[
                            :, :, :HD
                        ],
                        in0=tp.rearrange("p (h c) -> p h c", h=2),
                        in1=bv_rep.rearrange("p (h c) -> p h c", h=2),
                        op=mybir.AluOpType.add,
                    )
                    nc.vector.tensor_copy(
                        out=vhat[:, j, :].rearrange("p (h c) -> p h c", h=2)[
                            :, :, HD : HD + 1
                        ],
                        in_=ones2.rearrange("p (h c) -> p h c", h=2),
                    )

            # ---- attention (S^T layout), per head / query-block ----
            # S matmuls are 3-band row-packed: band a of the PE computes
            # m-chunk 3g+a concurrently (K=32 each). k_rep3/q_rep3 hold
            # band-shifted copies so one contiguous AP addresses all bands.
            k_rep3 = [
                singles.tile([96, N], F32R, name=f"krep{h}", tag=f"krep{h}")
                for h in range(HPC)
            ]
            q_rep3 = [
                singles.tile([96, N], F32R, name=f"qrep{h}", tag=f"qrep{h}")
                for h in range(HPC)
            ]
            for h in range(HPC):
                for a in range(3):
                    nc.sync.dma_start(
                        out=k_rep3[h][32 * a : 32 * a + 32, 0 : N - 128 * a],
                        in_=k_sb[HD * h : HD * h + 32, 128 * a : N],
                    )
                    nc.sync.dma_start(
                        out=q_rep3[h][32 * a : 32 * a + 32, :],
                        in_=q_sb[HD * h : HD * h + 32, :],
                    )

            # per-head O_aug (32 dims + den row), partition base 0
            oaug_h = [
                singles.tile([HD + 1, N], F32, name=f"oaug{h}", tag=f"oaug{h}")
                for h in range(HPC)
            ]
            NG = NCH // 3  # m-chunk groups of 3
            with (
                tc.tile_pool(name="spsum", bufs=2, space="PSUM") as spsum,
                tc.tile_pool(name="opsum", bufs=1, space="PSUM") as opsum,
            ):
                for h in range(HPC):
                    vh = slice(33 * h, 33 * h + 33)
                    for qoff, qln in _pieces(N, NQB):
                        o_ps = opsum.tile([HD + 1, NQB], F32, tag="o")
                        for g in range(NG):
                            s_tri = spsum.tile([128, 3 * NQB], F32, tag="s")
                            for a in range(3):
                                nc.tensor.matmul(
                                    s_tri[:, NQB * a : NQB * a + qln],
                                    k_rep3[h][
                                        32 * a : 32 * a + 32,
                                        384 * g : 384 * g + 128,
                                    ],
                                    q_rep3[h][
                                        32 * a : 32 * a + 32, qoff : qoff + qln
                                    ],
                                    start=True,
                                    stop=True,
                                )
                            ex = expp.tile([128, 3 * NQB], F32R, tag="ex")
                            nc.scalar.activation(
                                out=ex.rearrange("p (a c) -> p a c", a=3)[
                                    :, :, :qln
                                ],
                                in_=s_tri.rearrange("p (a c) -> p a c", a=3)[
                                    :, :, :qln
                                ],
                                func=mybir.ActivationFunctionType.Exp,
                                scale=SCALE,
                            )
                            for a in range(3):
                                nc.tensor.matmul(
                                    o_ps[:, :qln],
                                    vhat[:, 3 * g + a, vh],
                                    ex[:, NQB * a : NQB * a + qln],
                                    start=(g == 0 and a == 0),
                                    stop=(g == NG - 1 and a == 2),
                                )
                        nc.vector.tensor_copy(
                            out=oaug_h[h][:, qoff : qoff + qln],
                            in_=o_ps[:, :qln],
                        )

            # ---- denominators -> per-chunk reciprocal -> broadcast ----
            # SBUF partition<->free transposes are done via a DRAM bounce
            # (DMA APs cannot mix partition and free dims in one descriptor).
            den_t = singles.tile([128, 2 * NCH], F32)
            inv_t = singles.tile([128, 2 * NCH], F32)
            inv_bc = [
                singles.tile([HD, N], F32, name=f"invbc{h}", tag=f"invbc{h}")
                for h in range(HPC)
            ]
            with tc.tile_pool(name="dram", bufs=1, space="DRAM") as dramp:
                den_dram = dramp.tile([2, N], F32, tag="dend")
                inv_dram = dramp.tile([2, N], F32, tag="invd")
                for h in range(HPC):
                    nc.sync.dma_start(
                        out=den_dram[h : h + 1, :],
                        in_=oaug_h[h][HD : HD + 1, :],
                    )
                    nc.sync.dma_start(
                        out=den_t[:, NCH * h : NCH * (h + 1)],
                        in_=den_dram[h : h + 1, :].rearrange(
                            "o (j p) -> (o p) j", p=128
                        ),
                    )
                nc.vector.reciprocal(out=inv_t, in_=den_t)
                for h in range(HPC):
                    nc.sync.dma_start(
                        out=inv_dram[h : h + 1, :].rearrange(
                            "o (j p) -> (o p) j", p=128
                        ),
                        in_=inv_t[:, NCH * h : NCH * (h + 1)],
                    )
                    src = inv_dram[h : h + 1, :]
                    bc = bass.AP(
                        tensor=src.tensor,
                        offset=src.offset,
                        ap=[[0, HD]] + [list(d) for d in src.ap[1:]],
                    )
                    nc.sync.dma_start(out=inv_bc[h], in_=bc)

            # ---- normalize O: on = O_unnorm * inv_den ----
            on_h = [
                singles.tile([HD, N], F32R, name=f"on{h}", tag=f"on{h}")
                for h in range(HPC)
            ]
            for h in range(HPC):
                nc.vector.tensor_mul(
                    out=on_h[h],
                    in0=oaug_h[h][:HD, :],
                    in1=inv_bc[h],
                )

            # ---- output projection: out_t[n, co] = on^T @ wo_t + bo ----
            with tc.tile_pool(name="wopsum", bufs=4, space="PSUM") as wopsum:
                for j in range(NCH):
                    wp = wopsum.tile([128, C], F32, tag="wo")
                    for h in range(HPC):
                        nc.tensor.matmul(
                            wp,
                            on_h[h][:, 128 * j : 128 * (j + 1)],
                            wo_h[h],
                            start=(h == 0),
                            stop=(h == HPC - 1),
                        )
                    ot = outp.tile([128, C], F32, tag="ot")
                    nc.vector.tensor_add(out=ot, in0=wp, in1=bo_rep)
                    nc.sync.dma_start(
                        out=out_d[128 * j : 128 * (j + 1), :], in_=ot
                    )

    if split:
        _split_excess_waits(nc)
    return nc


def kernel(x, Wq, bq, Wk, bk, Wv, bv, Wo, bo):
    global LAST_RESULTS, _CACHED_NC
    x = np.ascontiguousarray(np.asarray(x, dtype=np.float32))
    Wq = np.asarray(Wq, dtype=np.float32)
    Wk = np.asarray(Wk, dtype=np.float32)
    Wv = np.asarray(Wv, dtype=np.float32)
    Wo = np.asarray(Wo, dtype=np.float32)
    bq = np.asarray(bq, dtype=np.float32)
    bk = np.asarray(bk, dtype=np.float32)
    bv = np.asarray(bv, dtype=np.float32)
    bo = np.asarray(bo, dtype=np.float32)

    xf = x.reshape(B, C, N)
    in_maps = []
    for core in range(NC):
        b = core // 4
        g = core % 4
        sl = slice(GD * g, GD * (g + 1))
        in_maps.append(
            {
                "x": np.ascontiguousarray(xf[b]),
                "wq_t": np.ascontiguousarray(Wq[sl, :].T),
                "wk_t": np.ascontiguousarray(Wk[sl, :].T),
                "wv_t": np.ascontiguousarray(Wv[sl, :].T),
                "bq": np.ascontiguousarray(bq[sl].reshape(GD, 1)),
                "bk": np.ascontiguousarray(bk[sl].reshape(GD, 1)),
                "bv": np.ascontiguousarray(bv[sl].reshape(GD, 1)),
                "wo_t": np.ascontiguousarray(Wo[:, sl].T),
                "bo": np.ascontiguousarray(
                    bo.reshape(C, 1) if g == 0 else np.zeros((C, 1), np.float32)
                ),
            }
        )

    if _CACHED_NC is None:
        _CACHED_NC = build_nc()
    res = run_bass_kernel_spmd(_CACHED_NC, in_maps, core_ids=list(range(NC)))
    LAST_RESULTS = res

    out = np.zeros((B, C, N), dtype=np.float32)
    for core in range(NC):
        out[core // 4] += res.results[core]["out_t"].T
    return out.reshape(B, C, 48, 48)
